# revision 9
# baseline (speedup 1.0000x reference)
"""GAT message-passing kernel for 8 Trainium2 NeuronCores.

Key algebraic property of the reference (faithful torch repeat_interleave
replication): with h = x @ proj_w.T + proj_b  [B, N, H],
    first[b, I, J, c]  = h[b, I, (J*H+c) // N] = h[b, I, J // (N//H)]
    second[b, I, J, c] = h[b, I, c]
so the pre-mask score collapses to
    scores[b, I, J] = leaky_relu(S1 * h[b, I, J//32] + d[b, I])
with S1 = sum(a_w[0, :H]) and d = h @ a_w[0, H:].  Each row of scores has
only H=32 distinct values (one per 32-column block of J).  Softmax+matmul
then reduce to a masked weighted aggregation that never materializes any
[N, N] tensor in HBM:
    W[b, I, J] = adj[I, J] * exp(leaky(v))[b, I, J//32]
    out[b, I, :] = (W @ h[b]) / rowsum(W)

Sharding: rows I are split 128-per-core across 8 cores (both batches on
every core).  dist_mat rows are sharded; x and the tiny weights are
replicated; every core redundantly computes full h (trivial FLOPs).

Device dataflow per core (c = core index, I in [128c, 128c+128)):
  - h_aug[J, m] for all 16 token tiles via paired PE transposes of x and
    block-diagonal weight matmuls (2 tiles per matmul), bias + ones
    column added during the PSUM->SBUF copy against a broadcast tile.
  - adjT[J, I] = (distT < thr) via paired PE transposes of the core's
    dist rows (diagonal pre-zeroed on host so the forced diag-1 holds).
  - eT[k, t] = exp(leaky(M32.T @ hT)) via small matmuls, spilled to DRAM
    and broadcast-replicated back as eb[J, I] = e[J//32, I] with strided
    replicate DMAs (no PE work).
  - W.T tile = adjT * eb (DVE), aggregated with PSUM-accumulated
    matmuls out_psum[I, m] += W.T_j.T @ h_aug_j, split over two PSUM
    banks to pipeline the fp32 passes; column 32 of h_aug is ones so
    column 32 accumulates Z (softmax denominator).  Final divide is a
    per-partition reciprocal+scale, DMA'd straight out.
The v-path ops are interleaved with the h-loop in program order so PE
never idles waiting on the serial cross-engine v chain.
"""

import sys

sys.path.insert(0, "/opt/trn_rl_repo")

import numpy as np

B, N, C, H = 2, 1024, 64, 32
P = 128                 # rows per core / partition tile
NCORES = 8
NT = B * N // P         # 16 token tiles of 128
NJ = N // P             # 8 column tiles of 128
NPAIR = NT // 2         # 8 paired token tiles
THR = 200000.0
ALPHA = 0.01
H1 = H + 1              # 33: h channels + ones column
H2 = 2 * H1             # 66: two tiles side by side

_CACHE = {}
LAST_RESULT = None


def _build():
    import concourse.bacc as bacc
    import concourse.bass as bass
    import concourse.tile as tile
    from concourse import masks, mybir

    F32 = mybir.dt.float32
    Alu = mybir.AluOpType
    Act = mybir.ActivationFunctionType

    nc = bacc.Bacc("TRN2", target_bir_lowering=False)

    xg_d = nc.dram_tensor("xg", (NT, P, C), F32, kind="ExternalInput")
    xo_d = nc.dram_tensor("xo", (B, P, C), F32, kind="ExternalInput")
    dist_d = nc.dram_tensor("dist", (P, N), F32, kind="ExternalInput")
    wt2_d = nc.dram_tensor("wt2", (P, H2), F32, kind="ExternalInput")
    brow2_d = nc.dram_tensor("brow2", (1, H2), F32, kind="ExternalInput")
    wta_d = nc.dram_tensor("wta", (C, H), F32, kind="ExternalInput")
    bcol_d = nc.dram_tensor("bcol", (H, 1), F32, kind="ExternalInput")
    m32_d = nc.dram_tensor("m32", (H, H), F32, kind="ExternalInput")
    out_d = nc.dram_tensor("out", (B, P, H), F32, kind="ExternalOutput")

    with tile.TileContext(nc) as tc:
        with (
            tc.tile_pool(name="const", bufs=1) as const,
            tc.tile_pool(name="persist", bufs=1) as persist,
            tc.tile_pool(name="work", bufs=3) as work,
            tc.tile_pool(name="dram", bufs=1, space="DRAM") as drampool,
            tc.tile_pool(name="psT", bufs=4, space="PSUM") as psT,
            tc.tile_pool(name="psA", bufs=2, space="PSUM") as psA,
        ):
            ident = const.tile([P, P], F32)
            masks.make_identity(nc, ident[:])

            # ---- input DMAs (issued up front, spread across both rings) ----
            xo_sb = persist.tile([P, B, C], F32)
            nc.sync.dma_start(out=xo_sb[:], in_=xo_d[:].rearrange("b p c -> p b c"))
            wt2 = const.tile([P, H2], F32)
            nc.scalar.dma_start(out=wt2[:], in_=wt2_d[:])
            wta = const.tile([C, H], F32)
            nc.scalar.dma_start(out=wta[:], in_=wta_d[:])
            bcol = const.tile([H, 1], F32)
            nc.scalar.dma_start(out=bcol[:], in_=bcol_d[:])
            m32 = const.tile([H, H], F32)
            nc.scalar.dma_start(out=m32[:], in_=m32_d[:])
            bias2 = const.tile([P, H2], F32)
            nc.scalar.dma_start(
                out=bias2[:],
                in_=bass.AP(tensor=brow2_d, offset=0, ap=[[0, P], [1, H2]]),
            )
            dist_sb = persist.tile([P, NJ, P], F32)
            dview = dist_d[:].rearrange("p (j q) -> p j q", j=NJ)
            nc.sync.dma_start(out=dist_sb[:, 0:4, :], in_=dview[:, 0:4, :])
            nc.scalar.dma_start(out=dist_sb[:, 4:8, :], in_=dview[:, 4:8, :])
            xg_sb = persist.tile([P, NT, C], F32)
            xview = xg_d[:].rearrange("g p c -> p g c")
            for q in range(4):
                eng = nc.sync if q % 2 == 0 else nc.scalar
                eng.dma_start(
                    out=xg_sb[:, 4 * q : 4 * q + 4, :],
                    in_=xview[:, 4 * q : 4 * q + 4, :],
                )

            h_all = persist.tile([P, NPAIR, H2], F32)
            adjT = persist.tile([P, NJ, P], F32)
            esc = drampool.tile([B, H, P], F32)
            eb_all = persist.tile([P, B, NJ, P], F32)

            # ---- interleaved: v-path (serial, latency-bound) + h-loop and
            # ---- adjT (dense PE work that hides the v-path stalls) ----
            def h_pair(p):
                ps_x = psT.tile([P, P], F32, tag="ps")
                nc.tensor.transpose(ps_x[:], xg_sb[:, 2 * p : 2 * p + 2, :], ident[:])
                xt2 = work.tile([P, P], F32, tag="xt2")
                nc.vector.tensor_copy(xt2[:], ps_x[:])
                ps_h = psT.tile([P, H2], F32, tag="ps")
                nc.tensor.matmul(ps_h[:], xt2[:], wt2[:])
                nc.vector.tensor_add(out=h_all[:, p, :], in0=ps_h[:], in1=bias2[:])

            def adj_pair(jp):
                ps_d = psT.tile([P, 2, P], F32, tag="ps")
                nc.tensor.transpose(ps_d[:, 0, :], dist_sb[:, 2 * jp, :], ident[:])
                nc.tensor.transpose(ps_d[:, 1, :], dist_sb[:, 2 * jp + 1, :], ident[:])
                nc.vector.tensor_scalar(
                    out=adjT[:, 2 * jp : 2 * jp + 2, :], in0=ps_d[:],
                    scalar1=THR, scalar2=None, op0=Alu.is_lt,
                )

            vstate = {}

            def v_step(b, step):
                if step == 0:
                    ps_xo = psT.tile([C, P], F32, tag="ps")
                    nc.tensor.transpose(ps_xo[:], xo_sb[:, b, :], ident[:])
                    xoa = work.tile([C, P], F32, tag="xoa")
                    nc.vector.tensor_copy(xoa[:], ps_xo[:])
                    vstate[b] = xoa
                elif step == 1:
                    ps_hT = psT.tile([H, P], F32, tag="ps")
                    nc.tensor.matmul(ps_hT[:], wta[:], vstate[b][:])
                    hToa = work.tile([H, P], F32, tag="hToa")
                    nc.vector.tensor_scalar(
                        out=hToa[:], in0=ps_hT[:], scalar1=bcol[:],
                        scalar2=None, op0=Alu.add,
                    )
                    vstate[b] = hToa
                elif step == 2:
                    ps_v = psT.tile([H, P], F32, tag="ps")
                    nc.tensor.matmul(ps_v[:], m32[:], vstate[b][:])
                    t1 = work.tile([H, P], F32, tag="t1")
                    nc.vector.tensor_scalar(
                        out=t1[:], in0=ps_v[:], scalar1=ALPHA,
                        scalar2=None, op0=Alu.mult,
                    )
                    t2 = work.tile([H, P], F32, tag="t2")
                    nc.vector.tensor_tensor(
                        out=t2[:], in0=ps_v[:], in1=t1[:], op=Alu.max
                    )
                    eT = work.tile([H, P], F32, tag="eT")
                    nc.scalar.activation(eT[:], t2[:], Act.Exp)
                    nc.sync.dma_start(out=esc[b], in_=eT[:])
                    for j in range(NJ):
                        src = esc[b, 4 * j : 4 * j + 4, :]
                        rep = bass.AP(
                            tensor=src.tensor,
                            offset=src.offset,
                            ap=[list(src.ap[0]), [0, H], list(src.ap[1])],
                        )
                        eng = nc.sync if j % 2 == 0 else nc.scalar
                        eng.dma_start(out=eb_all[:, b, j, :], in_=rep)

            # interleave schedule: PE-dense work between serial v-path hops
            v_step(0, 0)
            h_pair(0)
            v_step(0, 1)
            h_pair(1)
            v_step(0, 2)
            v_step(1, 0)
            h_pair(2)
            v_step(1, 1)
            h_pair(3)
            v_step(1, 2)
            h_pair(4)
            adj_pair(0)
            h_pair(5)
            adj_pair(1)
            h_pair(6)
            adj_pair(2)
            h_pair(7)
            adj_pair(3)

            # ---- masked weighted aggregation, two PSUM banks per batch ----
            for b in range(B):
                ps_even = psA.tile([P, H1], F32, tag="pse")
                ps_odd = psA.tile([P, H1], F32, tag="pso")
                for j in range(NJ):
                    wtile = work.tile([P, P], F32, tag="wtile")
                    nc.vector.tensor_mul(
                        wtile[:], adjT[:, j, :], eb_all[:, b, j, :]
                    )
                    g = b * NJ + j
                    ps_o = ps_even if j % 2 == 0 else ps_odd
                    nc.tensor.matmul(
                        ps_o[:],
                        wtile[:],
                        h_all[:, g // 2, H1 * (g % 2) : H1 * (g % 2) + H1],
                        start=(j < 2),
                        stop=(j >= NJ - 2),
                    )
                otmp = work.tile([P, H1], F32, tag="otmp")
                nc.vector.tensor_copy(otmp[:], ps_odd[:])
                osum = work.tile([P, H1], F32, tag="osum")
                nc.vector.tensor_add(out=osum[:], in0=ps_even[:], in1=otmp[:])
                zr = work.tile([P, 1], F32, tag="zr")
                nc.vector.reciprocal(zr[:], osum[:, H : H + 1])
                ot = work.tile([P, H], F32, tag="ot")
                nc.vector.tensor_scalar_mul(out=ot[:], in0=osum[:, 0:H], scalar1=zr[:])
                eng = nc.sync if b == 0 else nc.scalar
                eng.dma_start(out=out_d[b], in_=ot[:])

    nc.finalize()
    return nc


def kernel(x, dist_mat, proj_w, proj_b, a_w, trace=False):
    global LAST_RESULT
    from concourse.bass_utils import run_bass_kernel_spmd

    x = np.ascontiguousarray(np.asarray(x, dtype=np.float32))
    dist_mat = np.asarray(dist_mat, dtype=np.float32)
    proj_w = np.asarray(proj_w, dtype=np.float32)
    proj_b = np.asarray(proj_b, dtype=np.float32).reshape(H)
    a_w = np.asarray(a_w, dtype=np.float32).reshape(2 * H)

    if "nc" not in _CACHE:
        _CACHE["nc"] = _build()
    nc = _CACHE["nc"]

    # ---- host-side constant folding (all tiny) ----
    a1, a2 = a_w[:H], a_w[H:]
    s1 = np.float32(a1.sum(dtype=np.float32))
    # wT with a zero ones-column slot (col H), no bias row
    wt_nb = np.zeros((C, H1), np.float32)
    wt_nb[:, :H] = proj_w.T
    # block-diagonal weights: two token tiles per matmul
    wt2 = np.zeros((P, H2), np.float32)
    wt2[0:C, 0:H1] = wt_nb
    wt2[C:P, H1:H2] = wt_nb
    # broadcast bias row (+1.0 in the ones-column slots)
    b_aug = np.concatenate([proj_b, np.float32([1.0])])
    brow2 = np.concatenate([b_aug, b_aug]).reshape(1, H2)
    # hT path: plain transposed weights; bias as per-partition column
    wta = np.ascontiguousarray(proj_w.T)
    bcol = proj_b.reshape(H, 1)
    # v = S1*h + (h@a2): fold into one [32, 32] matrix against hT
    m32 = s1 * np.eye(H, dtype=np.float32) + a2[:, None]

    dist_fixed = dist_mat.copy()
    np.fill_diagonal(dist_fixed, 0.0)  # adj diagonal forced to 1

    xg = x.reshape(NT, P, C)
    in_maps = []
    for c in range(NCORES):
        sl = slice(c * P, (c + 1) * P)
        in_maps.append(
            {
                "xg": xg,
                "xo": np.ascontiguousarray(x[:, sl, :]),
                "dist": dist_fixed[sl],
                "wt2": wt2,
                "brow2": brow2,
                "wta": wta,
                "bcol": bcol,
                "m32": m32,
            }
        )

    res = run_bass_kernel_spmd(nc, in_maps, core_ids=list(range(NCORES)), trace=trace)
    LAST_RESULT = res
    return np.concatenate([res.results[c]["out"] for c in range(NCORES)], axis=1)


# revision 11
# speedup vs baseline: 1.1216x; 1.1216x over previous
"""GAT message-passing kernel for 8 Trainium2 NeuronCores.

Key algebraic property of the reference (faithful torch repeat_interleave
replication): with h = x @ proj_w.T + proj_b  [B, N, H],
    first[b, I, J, c]  = h[b, I, (J*H+c) // N] = h[b, I, J // (N//H)]
    second[b, I, J, c] = h[b, I, c]
so the pre-mask score collapses to
    scores[b, I, J] = leaky_relu(S1 * h[b, I, J//32] + d[b, I])
with S1 = sum(a_w[0, :H]) and d = h @ a_w[0, H:].  Each row of scores has
only H=32 distinct values (one per 32-column block of J).  Softmax+matmul
then reduce to a masked weighted aggregation that never materializes any
[N, N] tensor in HBM:
    W[b, I, J] = adj[I, J] * exp(leaky(v))[b, I, J//32]
    out[b, I, :] = (W @ h[b]) / rowsum(W)

Sharding: rows I are split 128-per-core across 8 cores (both batches on
every core).  dist_mat rows are sharded; x and the tiny weights are
replicated; every core redundantly computes full h (trivial FLOPs).

J-side tiling trick: neighbor tokens are enumerated as J = 8*q + tk
(q = partition, tk = tile index 0..7).  Then
  - x loads are fully contiguous (each SBUF partition reads one 2KB row),
  - every J-tile has k(J) = J//32 = q//4, so the broadcast weight tile
    eb[q, I] = e[q//4, I] is THE SAME for all 8 J-tiles: one SBUF->SBUF
    replicate DMA per batch replaces all broadcast matmuls,
  - adjT tiles come from PE transposes of stride-8 column slices of the
    core's dist rows (diagonal pre-zeroed on host).
h_aug is built with paired PE transposes + block-diagonal weight matmuls
(2 token tiles per matmul); bias and the ones column are added during the
PSUM->SBUF copy against a broadcast bias tile.  The aggregation
out_psum[I, m] += W.T_tk.T @ h_aug_tk accumulates over two PSUM banks to
pipeline fp32 passes; column 32 of h_aug is ones so column 32 holds the
softmax denominator Z.  Emission order interleaves the serial v-path and
the aggregation with the dense h/adj PE work so PE never idles.
"""

import sys

sys.path.insert(0, "/opt/trn_rl_repo")

import numpy as np

B, N, C, H = 2, 1024, 64, 32
P = 128                 # rows per core / partition tile
NCORES = 8
NJ = N // P             # 8 J-tiles of 128
THR = 200000.0
ALPHA = 0.01
H1 = H + 1              # 33: h channels + ones column
H2 = 2 * H1             # 66: two tiles side by side

_CACHE = {}
LAST_RESULT = None


def _build():
    import concourse.bacc as bacc
    import concourse.bass as bass
    import concourse.tile as tile
    from concourse import masks, mybir

    F32 = mybir.dt.float32
    Alu = mybir.AluOpType
    Act = mybir.ActivationFunctionType

    nc = bacc.Bacc("TRN2", target_bir_lowering=False)

    xg_d = nc.dram_tensor("xg", (B, P, NJ * C), F32, kind="ExternalInput")
    xo_d = nc.dram_tensor("xo", (B, P, C), F32, kind="ExternalInput")
    dist_d = nc.dram_tensor("dist", (P, N), F32, kind="ExternalInput")
    wt2_d = nc.dram_tensor("wt2", (P, H2), F32, kind="ExternalInput")
    brow2_d = nc.dram_tensor("brow2", (1, H2), F32, kind="ExternalInput")
    wta_d = nc.dram_tensor("wta", (C, H), F32, kind="ExternalInput")
    bcol_d = nc.dram_tensor("bcol", (H, 1), F32, kind="ExternalInput")
    m32_d = nc.dram_tensor("m32", (H, H), F32, kind="ExternalInput")
    out_d = nc.dram_tensor("out", (B, P, H), F32, kind="ExternalOutput")

    with tile.TileContext(nc) as tc:
        with (
            tc.tile_pool(name="const", bufs=1) as const,
            tc.tile_pool(name="persist", bufs=1) as persist,
            tc.tile_pool(name="work", bufs=3) as work,
            tc.tile_pool(name="psT", bufs=4, space="PSUM") as psT,
            tc.tile_pool(name="psA", bufs=2, space="PSUM") as psA,
        ):
            ident = const.tile([P, P], F32)
            masks.make_identity(nc, ident[:])

            # ---- input DMAs: all large loads are fully contiguous ----
            xo_sb = persist.tile([P, B, C], F32)
            nc.sync.dma_start(out=xo_sb[:], in_=xo_d[:].rearrange("b p c -> p b c"))
            xg_sb = persist.tile([P, B, NJ * C], F32)
            nc.sync.dma_start(out=xg_sb[:, 0, :], in_=xg_d[0])
            nc.scalar.dma_start(out=xg_sb[:, 1, :], in_=xg_d[1])
            dist_sb = persist.tile([P, N], F32)
            nc.sync.dma_start(out=dist_sb[:], in_=dist_d[:])
            wt2 = const.tile([P, H2], F32)
            nc.scalar.dma_start(out=wt2[:], in_=wt2_d[:])
            wta = const.tile([C, H], F32)
            nc.scalar.dma_start(out=wta[:], in_=wta_d[:])
            bcol = const.tile([H, 1], F32)
            nc.scalar.dma_start(out=bcol[:], in_=bcol_d[:])
            m32 = const.tile([H, H], F32)
            nc.scalar.dma_start(out=m32[:], in_=m32_d[:])
            bias2 = const.tile([P, H2], F32)
            nc.scalar.dma_start(
                out=bias2[:],
                in_=bass.AP(tensor=brow2_d, offset=0, ap=[[0, P], [1, H2]]),
            )

            h_all = persist.tile([P, B, NJ // 2, H2], F32)
            adjT = persist.tile([P, NJ, P], F32)
            eb_all = persist.tile([P, B, P], F32)
            dview = dist_sb[:].rearrange("p (q t) -> p q t", t=NJ)

            vstate = {}

            def v_step(b, step):
                if step == 0:
                    ps_xo = psT.tile([C, P], F32, tag="ps")
                    nc.tensor.transpose(ps_xo[:], xo_sb[:, b, :], ident[:])
                    xoa = work.tile([C, P], F32, tag="xoa")
                    nc.vector.tensor_copy(xoa[:], ps_xo[:])
                    vstate[b] = xoa
                elif step == 1:
                    ps_hT = psT.tile([H, P], F32, tag="ps")
                    nc.tensor.matmul(ps_hT[:], wta[:], vstate[b][:])
                    hToa = work.tile([H, P], F32, tag="hToa")
                    nc.vector.tensor_scalar(
                        out=hToa[:], in0=ps_hT[:], scalar1=bcol[:],
                        scalar2=None, op0=Alu.add,
                    )
                    vstate[b] = hToa
                else:
                    ps_v = psT.tile([H, P], F32, tag="ps")
                    nc.tensor.matmul(ps_v[:], m32[:], vstate[b][:])
                    t1 = work.tile([H, P], F32, tag="t1")
                    nc.vector.tensor_scalar(
                        out=t1[:], in0=ps_v[:], scalar1=ALPHA,
                        scalar2=None, op0=Alu.mult,
                    )
                    t2 = work.tile([H, P], F32, tag="t2")
                    nc.vector.tensor_tensor(
                        out=t2[:], in0=ps_v[:], in1=t1[:], op=Alu.max
                    )
                    eT = work.tile([H, P], F32, tag="eT")
                    nc.scalar.activation(eT[:], t2[:], Act.Exp)
                    # one replicate DMA: eb[q, I] = e[q//4, I] for every tile
                    src = eT[:]
                    rep = bass.AP(
                        tensor=src.tensor, offset=src.offset,
                        ap=[list(src.ap[0]), [0, 4], list(src.ap[1])],
                    )
                    eng = nc.sync if b == 0 else nc.scalar
                    eng.dma_start(out=eb_all[:, b, :], in_=rep)

            def adj_pair(u):
                ps_d = psT.tile([P, 2, P], F32, tag="ps")
                nc.tensor.transpose(ps_d[:, 0, :], dview[:, :, 2 * u], ident[:])
                nc.tensor.transpose(ps_d[:, 1, :], dview[:, :, 2 * u + 1], ident[:])
                nc.vector.tensor_scalar(
                    out=adjT[:, 2 * u : 2 * u + 2, :], in0=ps_d[:],
                    scalar1=THR, scalar2=None, op0=Alu.is_lt,
                )

            def h_pair(b, u):
                ps_x = psT.tile([P, P], F32, tag="ps")
                nc.tensor.transpose(
                    ps_x[:], xg_sb[:, b, P * u : P * u + P], ident[:]
                )
                xt2 = work.tile([P, P], F32, tag="xt2")
                nc.vector.tensor_copy(xt2[:], ps_x[:])
                ps_h = psT.tile([P, H2], F32, tag="ps")
                nc.tensor.matmul(ps_h[:], xt2[:], wt2[:])
                nc.vector.tensor_add(out=h_all[:, b, u, :], in0=ps_h[:], in1=bias2[:])

            aggst = {}

            def agg_step(b, tk):
                if tk == 0:
                    aggst[b] = (
                        psA.tile([P, H1], F32, tag="pse", name=f"pse{b}"),
                        psA.tile([P, H1], F32, tag="pso", name=f"pso{b}"),
                    )
                wtile = work.tile([P, P], F32, tag="wtile")
                nc.vector.tensor_mul(wtile[:], adjT[:, tk, :], eb_all[:, b, :])
                ps_o = aggst[b][tk % 2]
                nc.tensor.matmul(
                    ps_o[:],
                    wtile[:],
                    h_all[:, b, tk // 2, H1 * (tk % 2) : H1 * (tk % 2) + H1],
                    start=(tk < 2),
                    stop=(tk >= NJ - 2),
                    skip_group_check=True,
                )

            def finalize(b):
                ps_even, ps_odd = aggst[b]
                otmp = work.tile([P, H1], F32, tag="otmp")
                nc.vector.tensor_copy(otmp[:], ps_odd[:])
                osum = work.tile([P, H1], F32, tag="osum")
                nc.vector.tensor_add(out=osum[:], in0=ps_even[:], in1=otmp[:])
                zr = work.tile([P, 1], F32, tag="zr")
                nc.vector.reciprocal(zr[:], osum[:, H : H + 1])
                ot = work.tile([P, H], F32, tag="ot")
                nc.vector.tensor_scalar_mul(out=ot[:], in0=osum[:, 0:H], scalar1=zr[:])
                eng = nc.sync if b == 0 else nc.scalar
                eng.dma_start(out=out_d[b], in_=ot[:])

            # ---- interleaved emission ----
            v_step(0, 0)
            v_step(1, 0)
            adj_pair(0)
            v_step(0, 1)
            adj_pair(1)
            v_step(1, 1)
            adj_pair(2)
            v_step(0, 2)
            adj_pair(3)
            v_step(1, 2)
            for u in range(4):
                h_pair(0, u)
            agg_step(0, 0)
            agg_step(0, 1)
            agg_step(0, 2)
            agg_step(0, 3)
            h_pair(1, 0)
            agg_step(0, 4)
            agg_step(0, 5)
            agg_step(0, 6)
            agg_step(0, 7)
            h_pair(1, 1)
            finalize(0)
            h_pair(1, 2)
            agg_step(1, 0)
            agg_step(1, 1)
            agg_step(1, 2)
            agg_step(1, 3)
            h_pair(1, 3)
            agg_step(1, 4)
            agg_step(1, 5)
            agg_step(1, 6)
            agg_step(1, 7)
            finalize(1)

    nc.finalize()
    return nc


def kernel(x, dist_mat, proj_w, proj_b, a_w, trace=False):
    global LAST_RESULT
    from concourse.bass_utils import run_bass_kernel_spmd

    x = np.ascontiguousarray(np.asarray(x, dtype=np.float32))
    dist_mat = np.asarray(dist_mat, dtype=np.float32)
    proj_w = np.asarray(proj_w, dtype=np.float32)
    proj_b = np.asarray(proj_b, dtype=np.float32).reshape(H)
    a_w = np.asarray(a_w, dtype=np.float32).reshape(2 * H)

    if "nc" not in _CACHE:
        _CACHE["nc"] = _build()
    nc = _CACHE["nc"]

    # ---- host-side constant folding (all tiny) ----
    a1, a2 = a_w[:H], a_w[H:]
    s1 = np.float32(a1.sum(dtype=np.float32))
    # wT with a zero ones-column slot (col H)
    wt_nb = np.zeros((C, H1), np.float32)
    wt_nb[:, :H] = proj_w.T
    # block-diagonal weights: two token tiles per matmul
    wt2 = np.zeros((P, H2), np.float32)
    wt2[0:C, 0:H1] = wt_nb
    wt2[C:P, H1:H2] = wt_nb
    # broadcast bias row (+1.0 in the ones-column slots)
    b_aug = np.concatenate([proj_b, np.float32([1.0])])
    brow2 = np.concatenate([b_aug, b_aug]).reshape(1, H2)
    # hT path: plain transposed weights; bias as per-partition column
    wta = np.ascontiguousarray(proj_w.T)
    bcol = proj_b.reshape(H, 1)
    # v = S1*h + (h@a2): fold into one [32, 32] matrix against hT
    m32 = s1 * np.eye(H, dtype=np.float32) + a2[:, None]

    dist_fixed = dist_mat.copy()
    np.fill_diagonal(dist_fixed, 0.0)  # adj diagonal forced to 1

    # token j = 8*q + tk: row q of xg[b] holds tokens 8q..8q+7 contiguously
    xg = x.reshape(B, P, NJ * C)
    in_maps = []
    for c in range(NCORES):
        sl = slice(c * P, (c + 1) * P)
        in_maps.append(
            {
                "xg": xg,
                "xo": np.ascontiguousarray(x[:, sl, :]),
                "dist": dist_fixed[sl],
                "wt2": wt2,
                "brow2": brow2,
                "wta": wta,
                "bcol": bcol,
                "m32": m32,
            }
        )

    res = run_bass_kernel_spmd(nc, in_maps, core_ids=list(range(NCORES)), trace=trace)
    LAST_RESULT = res
    return np.concatenate([res.results[c]["out"] for c in range(NCORES)], axis=1)


# revision 14
# speedup vs baseline: 1.1977x; 1.0679x over previous
"""GAT message-passing kernel for 8 Trainium2 NeuronCores.

Key algebraic property of the reference (faithful torch repeat_interleave
replication): with h = x @ proj_w.T + proj_b  [B, N, H],
    first[b, I, J, c]  = h[b, I, (J*H+c) // N] = h[b, I, J // (N//H)]
    second[b, I, J, c] = h[b, I, c]
so the pre-mask score collapses to
    scores[b, I, J] = leaky_relu(S1 * h[b, I, J//32] + d[b, I])
with S1 = sum(a_w[0, :H]) and d = h @ a_w[0, H:].  Each row of scores has
only H=32 distinct values (one per 32-column block of J).  Softmax+matmul
then reduce to a masked weighted aggregation that never materializes any
[N, N] tensor in HBM:
    W[b, I, J] = adj[I, J] * exp(leaky(v))[b, I, J//32]
    out[b, I, :] = (W @ h[b]) / rowsum(W)

Sharding: rows I are split 128-per-core across 8 cores (both batches on
every core).  dist_mat rows are sharded; x and the tiny weights are
replicated; every core redundantly computes full h (trivial FLOPs).

J-side tiling trick: neighbor tokens are enumerated as J = 8*q + tk
(q = partition, tk = tile index 0..7).  Then
  - x loads are fully contiguous (each SBUF partition reads one 2KB row),
  - every J-tile has k(J) = J//32 = q//4, so the broadcast weight tile
    eb[q, I] = e[q//4, I] is THE SAME for all 8 J-tiles: one SBUF->SBUF
    replicate DMA per batch replaces all broadcast matmuls,
  - adjT tiles come from PE transposes of stride-8 column slices of the
    core's dist rows (diagonal pre-zeroed on host).
h_aug is built with paired PE transposes + block-diagonal weight matmuls
(2 token tiles per matmul); bias and the ones column are added during the
PSUM->SBUF copy against a broadcast bias tile.  The aggregation
out_psum[I, m] += W.T_tk.T @ h_aug_tk accumulates over two PSUM banks to
pipeline fp32 passes; column 32 of h_aug is ones so column 32 holds the
softmax denominator Z.  Emission order interleaves the serial v-path and
the aggregation with the dense h/adj PE work so PE never idles.
"""

import sys

sys.path.insert(0, "/opt/trn_rl_repo")

import numpy as np

B, N, C, H = 2, 1024, 64, 32
P = 128                 # rows per core / partition tile
NCORES = 8
NJ = N // P             # 8 J-tiles of 128
THR = 200000.0
ALPHA = 0.01
H1 = H + 1              # 33: h channels + ones column
H2 = 2 * H1             # 66: two tiles side by side

_CACHE = {}
LAST_RESULT = None


def _build():
    import concourse.bacc as bacc
    import concourse.bass as bass
    import concourse.tile as tile
    from concourse import masks, mybir

    F32 = mybir.dt.float32
    Alu = mybir.AluOpType
    Act = mybir.ActivationFunctionType

    nc = bacc.Bacc("TRN2", target_bir_lowering=False)

    xg_d = nc.dram_tensor("xg", (B, P, NJ * C), F32, kind="ExternalInput")
    xo_d = nc.dram_tensor("xo", (B, P, C), F32, kind="ExternalInput")
    dist_d = nc.dram_tensor("dist", (P, N), F32, kind="ExternalInput")
    wt2_d = nc.dram_tensor("wt2", (P, H2), F32, kind="ExternalInput")
    brow2_d = nc.dram_tensor("brow2", (1, H2), F32, kind="ExternalInput")
    wta_d = nc.dram_tensor("wta", (C, H), F32, kind="ExternalInput")
    bcol_d = nc.dram_tensor("bcol", (H, 1), F32, kind="ExternalInput")
    m32_d = nc.dram_tensor("m32", (H, H), F32, kind="ExternalInput")
    out_d = nc.dram_tensor("out", (B, P, H), F32, kind="ExternalOutput")

    with tile.TileContext(nc) as tc:
        with (
            tc.tile_pool(name="const", bufs=1) as const,
            tc.tile_pool(name="persist", bufs=1) as persist,
            tc.tile_pool(name="work", bufs=3) as work,
            tc.tile_pool(name="psT", bufs=4, space="PSUM") as psT,
            tc.tile_pool(name="psA", bufs=2, space="PSUM") as psA,
        ):
            ident = const.tile([P, P], F32)
            masks.make_identity(nc, ident[:])

            # ---- input DMAs: all large loads are fully contiguous ----
            xo_sb = persist.tile([P, B, C], F32)
            nc.sync.dma_start(out=xo_sb[:], in_=xo_d[:].rearrange("b p c -> p b c"))
            dist_sb = persist.tile([P, N], F32)
            nc.sync.dma_start(out=dist_sb[:, 0:512], in_=dist_d[:, 0:512])
            nc.scalar.dma_start(out=dist_sb[:, 512:1024], in_=dist_d[:, 512:1024])
            xg_sb = persist.tile([P, B, NJ * C], F32)
            nc.sync.dma_start(out=xg_sb[:, 0, :], in_=xg_d[0])
            nc.scalar.dma_start(out=xg_sb[:, 1, :], in_=xg_d[1])
            wt2 = const.tile([P, H2], F32)
            nc.scalar.dma_start(out=wt2[:], in_=wt2_d[:])
            wta = const.tile([C, H], F32)
            nc.scalar.dma_start(out=wta[:], in_=wta_d[:])
            bcol = const.tile([H, 1], F32)
            nc.scalar.dma_start(out=bcol[:], in_=bcol_d[:])
            m32 = const.tile([H, H], F32)
            nc.scalar.dma_start(out=m32[:], in_=m32_d[:])
            bias2 = const.tile([P, H2], F32)
            nc.scalar.dma_start(
                out=bias2[:],
                in_=bass.AP(tensor=brow2_d, offset=0, ap=[[0, P], [1, H2]]),
            )

            h_all = persist.tile([P, B, NJ // 2, H2], F32)
            adjT = persist.tile([P, NJ, P], F32)
            eb_all = persist.tile([P, B, P], F32)
            dview = dist_sb[:].rearrange("p (q t) -> p q t", t=NJ)

            vstate = {}

            def v_step(b, step):
                if step == 0:
                    ps_xo = psT.tile([C, P], F32, tag="ps")
                    nc.tensor.transpose(ps_xo[:], xo_sb[:, b, :], ident[:])
                    xoa = work.tile([C, P], F32, tag="xoa")
                    nc.vector.tensor_copy(xoa[:], ps_xo[:])
                    vstate[b] = xoa
                elif step == 1:
                    ps_hT = psT.tile([H, P], F32, tag="ps")
                    nc.tensor.matmul(ps_hT[:], wta[:], vstate[b][:])
                    hToa = work.tile([H, P], F32, tag="hToa")
                    nc.vector.tensor_scalar(
                        out=hToa[:], in0=ps_hT[:], scalar1=bcol[:],
                        scalar2=None, op0=Alu.add,
                    )
                    vstate[b] = hToa
                else:
                    ps_v = psT.tile([H, P], F32, tag="ps")
                    nc.tensor.matmul(ps_v[:], m32[:], vstate[b][:])
                    t1 = work.tile([H, P], F32, tag="t1")
                    nc.vector.tensor_scalar(
                        out=t1[:], in0=ps_v[:], scalar1=ALPHA,
                        scalar2=None, op0=Alu.mult,
                    )
                    t2 = work.tile([H, P], F32, tag="t2")
                    nc.vector.tensor_tensor(
                        out=t2[:], in0=ps_v[:], in1=t1[:], op=Alu.max
                    )
                    eT = work.tile([H, P], F32, tag="eT")
                    nc.scalar.activation(eT[:], t2[:], Act.Exp)
                    # one replicate DMA: eb[q, I] = e[q//4, I] for every tile
                    src = eT[:]
                    rep = bass.AP(
                        tensor=src.tensor, offset=src.offset,
                        ap=[list(src.ap[0]), [0, 4], list(src.ap[1])],
                    )
                    eng = nc.sync if b == 0 else nc.scalar
                    eng.dma_start(out=eb_all[:, b, :], in_=rep)

            def adj_pair(u):
                ps_d = psT.tile([P, 2, P], F32, tag="ps")
                nc.tensor.transpose(ps_d[:, 0, :], dview[:, :, 2 * u], ident[:])
                nc.tensor.transpose(ps_d[:, 1, :], dview[:, :, 2 * u + 1], ident[:])
                nc.vector.tensor_scalar(
                    out=adjT[:, 2 * u : 2 * u + 2, :], in0=ps_d[:],
                    scalar1=THR, scalar2=None, op0=Alu.is_lt,
                )

            hstate = {}

            def h_T(b, u):
                ps_x = psT.tile([P, P], F32, tag="ps")
                nc.tensor.transpose(
                    ps_x[:], xg_sb[:, b, P * u : P * u + P], ident[:]
                )
                xt2 = work.tile([P, P], F32, tag="xt2")
                nc.vector.tensor_copy(xt2[:], ps_x[:])
                hstate[(b, u)] = xt2

            def h_M(b, u):
                ps_h = psT.tile([P, H2], F32, tag="ps")
                nc.tensor.matmul(ps_h[:], hstate[(b, u)][:], wt2[:])
                nc.vector.tensor_add(out=h_all[:, b, u, :], in0=ps_h[:], in1=bias2[:])

            aggst = {}

            def agg_step(b, tk):
                if tk == 0:
                    aggst[b] = (
                        psA.tile([P, H1], F32, tag="pse", name=f"pse{b}"),
                        psA.tile([P, H1], F32, tag="pso", name=f"pso{b}"),
                    )
                wtile = work.tile([P, P], F32, tag="wtile")
                nc.vector.tensor_mul(wtile[:], adjT[:, tk, :], eb_all[:, b, :])
                ps_o = aggst[b][tk % 2]
                nc.tensor.matmul(
                    ps_o[:],
                    wtile[:],
                    h_all[:, b, tk // 2, H1 * (tk % 2) : H1 * (tk % 2) + H1],
                    start=(tk < 2),
                    stop=(tk >= NJ - 2),
                    skip_group_check=True,
                )

            def finalize(b):
                ps_even, ps_odd = aggst[b]
                otmp = work.tile([P, H1], F32, tag="otmp")
                nc.vector.tensor_copy(otmp[:], ps_odd[:])
                osum = work.tile([P, H1], F32, tag="osum")
                nc.vector.tensor_add(out=osum[:], in0=ps_even[:], in1=otmp[:])
                zr = work.tile([P, 1], F32, tag="zr")
                nc.vector.reciprocal(zr[:], osum[:, H : H + 1])
                ot = work.tile([P, H], F32, tag="ot")
                nc.vector.tensor_scalar_mul(out=ot[:], in0=osum[:, 0:H], scalar1=zr[:])
                eng = nc.sync if b == 0 else nc.scalar
                eng.dma_start(out=out_d[b], in_=ot[:])

            # ---- interleaved emission ----
            v_step(0, 0)
            v_step(1, 0)
            adj_pair(0)
            v_step(0, 1)
            adj_pair(1)
            v_step(1, 1)
            adj_pair(2)
            v_step(0, 2)
            adj_pair(3)
            v_step(1, 2)
            h_T(0, 0)
            h_T(0, 1)
            h_M(0, 0)
            h_T(0, 2)
            h_M(0, 1)
            h_T(0, 3)
            h_M(0, 2)
            agg_step(0, 0)
            agg_step(0, 1)
            h_M(0, 3)
            agg_step(0, 2)
            agg_step(0, 3)
            h_T(1, 0)
            agg_step(0, 4)
            agg_step(0, 5)
            h_M(1, 0)
            h_T(1, 1)
            agg_step(0, 6)
            agg_step(0, 7)
            h_M(1, 1)
            finalize(0)
            h_T(1, 2)
            agg_step(1, 0)
            agg_step(1, 1)
            h_M(1, 2)
            h_T(1, 3)
            agg_step(1, 2)
            agg_step(1, 3)
            h_M(1, 3)
            agg_step(1, 4)
            agg_step(1, 5)
            agg_step(1, 6)
            agg_step(1, 7)
            finalize(1)

    nc.finalize()
    return nc


def kernel(x, dist_mat, proj_w, proj_b, a_w, trace=False):
    global LAST_RESULT
    from concourse.bass_utils import run_bass_kernel_spmd

    x = np.ascontiguousarray(np.asarray(x, dtype=np.float32))
    dist_mat = np.asarray(dist_mat, dtype=np.float32)
    proj_w = np.asarray(proj_w, dtype=np.float32)
    proj_b = np.asarray(proj_b, dtype=np.float32).reshape(H)
    a_w = np.asarray(a_w, dtype=np.float32).reshape(2 * H)

    if "nc" not in _CACHE:
        _CACHE["nc"] = _build()
    nc = _CACHE["nc"]

    # ---- host-side constant folding (all tiny) ----
    a1, a2 = a_w[:H], a_w[H:]
    s1 = np.float32(a1.sum(dtype=np.float32))
    # wT with a zero ones-column slot (col H)
    wt_nb = np.zeros((C, H1), np.float32)
    wt_nb[:, :H] = proj_w.T
    # block-diagonal weights: two token tiles per matmul
    wt2 = np.zeros((P, H2), np.float32)
    wt2[0:C, 0:H1] = wt_nb
    wt2[C:P, H1:H2] = wt_nb
    # broadcast bias row (+1.0 in the ones-column slots)
    b_aug = np.concatenate([proj_b, np.float32([1.0])])
    brow2 = np.concatenate([b_aug, b_aug]).reshape(1, H2)
    # hT path: plain transposed weights; bias as per-partition column
    wta = np.ascontiguousarray(proj_w.T)
    bcol = proj_b.reshape(H, 1)
    # v = S1*h + (h@a2): fold into one [32, 32] matrix against hT
    m32 = s1 * np.eye(H, dtype=np.float32) + a2[:, None]

    dist_fixed = dist_mat.copy()
    np.fill_diagonal(dist_fixed, 0.0)  # adj diagonal forced to 1

    # token j = 8*q + tk: row q of xg[b] holds tokens 8q..8q+7 contiguously
    xg = x.reshape(B, P, NJ * C)
    in_maps = []
    for c in range(NCORES):
        sl = slice(c * P, (c + 1) * P)
        in_maps.append(
            {
                "xg": xg,
                "xo": np.ascontiguousarray(x[:, sl, :]),
                "dist": dist_fixed[sl],
                "wt2": wt2,
                "brow2": brow2,
                "wta": wta,
                "bcol": bcol,
                "m32": m32,
            }
        )

    res = run_bass_kernel_spmd(nc, in_maps, core_ids=list(range(NCORES)), trace=trace)
    LAST_RESULT = res
    return np.concatenate([res.results[c]["out"] for c in range(NCORES)], axis=1)
